# revision 1
# baseline (speedup 1.0000x reference)
# kernel.py — MoE (E=16, top-4) Trainium2 Bass kernel, expert-parallel over 8 cores.
#
# Strategy:
#   - Router (Linear->ReLU->Linear, top-4 softmax) computed data-parallel in fp32:
#     each core handles a 256-token shard, AllGather of the dense combine weights.
#   - The expert load is highly skewed (ReLU features + random router -> a few
#     experts take most tokens), so compute is organized as NSLOT=3 "slots" per
#     core with host-planned capacities: the host runs the router (only to pick
#     static shapes / slot assignment), splits heavy experts into position
#     ranges, and packs (expert, pos-range) pieces into 24 uniform-per-position
#     slots. The device recomputes routing exactly and dispatches by itself.
#   - Dispatch per slot: expert mask -> exclusive cumsum (PE matmuls against
#     triangular constants) -> gate to [lo, lo+cap) -> indirect-DMA scatter of
#     token ids -> gather of routed token rows.
#   - Expert MLP on routed tokens with float32r matmuls (full-rate PE):
#     h^T = gelu(W1e^T x^T + b1), y = (h W2e + b2) * combine, indirect-DMA
#     scatter-add into a dense [T,H] accumulator.
#   - ReduceScatter(add) over 8 cores; each core emits its 256-token shard;
#     host concatenates shards.
import numpy as np

H = 1024
F = 4096
E = 16
TOPK = 4
T = 2048
NCORES = 8
TSH = T // NCORES          # 256 router tokens per core
DUMP = T                   # dump token row index (row T of the [T+1] buffers)
NEG = -3.0e38
MARGIN = 32                # slack over host-computed counts (host/device drift)

_CACHE = {}


# ---------------------------------------------------------------------------
# Host-side planning: counts -> slot capacities + (expert, lo) assignment
# ---------------------------------------------------------------------------

def _host_counts(inputs):
    x = np.asarray(inputs["x"], np.float32).reshape(T, H)
    h = np.maximum(x @ np.asarray(inputs["Wr1"], np.float32)
                   + np.asarray(inputs["br1"], np.float32), 0.0)
    lg = h @ np.asarray(inputs["Wr2"], np.float32) + np.asarray(inputs["br2"], np.float32)
    order = np.argsort(-lg, axis=1, kind="stable")[:, :TOPK]
    counts = np.zeros(E, np.int64)
    for e in range(E):
        counts[e] = (order == e).sum()
    return counts


def _try_pack(counts, caps):
    """Greedy-pack experts (split into position ranges) into 8 slots per
    capacity position. Returns list of (expert, lo) per (position, coreslot)
    or None if infeasible."""
    avail = {a: 8 for a in caps}
    sizes = sorted(set(caps), reverse=True)
    pieces = []  # (cap, expert, lo)
    for e in np.argsort(-counts):
        rem = int(counts[e]) + MARGIN
        lo = 0
        while rem > 0:
            pick = None
            for a in reversed(sizes):  # best fit: smallest cap >= rem finishes it
                if avail.get(a, 0) > 0 and a >= rem:
                    pick = a
                    break
            if pick is None:           # otherwise: largest available, keep going
                for a in sizes:
                    if avail.get(a, 0) > 0:
                        pick = a
                        break
            if pick is None:
                return None
            avail[pick] -= 1
            pieces.append((pick, int(e), lo))
            lo += pick
            rem -= pick
    # distribute pieces to (position, core) honoring capacity positions
    slots = {a: [] for a in set(caps)}
    for cap, e, lo in pieces:
        slots[cap].append((e, lo))
    out = []
    used = {a: 0 for a in set(caps)}
    for a in caps:
        pos = []
        for r in range(NCORES):
            i = used[a]
            if i < len(slots[a]):
                pos.append(slots[a][i])
            else:
                pos.append((0, T + 4096))   # empty slot: range never matches
            used[a] += 1
        out.append(pos)
    return out


def _plan(inputs):
    counts = _host_counts(inputs)
    cands = []
    opts = [128, 256, 384, 512, 640]
    for n in (3, 4):
        def rec(pref):
            if len(pref) == n:
                cands.append(tuple(pref))
                return
            for a in opts:
                if not pref or a <= pref[-1]:
                    rec(pref + [a])
        rec([])
    cands.sort(key=lambda c: (len(c), sum(c)))
    for caps in cands:
        asg = _try_pack(counts, list(caps))
        if asg is not None:
            return list(caps), asg
    raise RuntimeError(f"no feasible slot packing for counts {counts}")


# ---------------------------------------------------------------------------
# Device program
# ---------------------------------------------------------------------------

def _build(caps, mm_dt_name="float32r"):
    import concourse.bass as bass
    import concourse.mybir as mybir
    import concourse.tile as tile
    from concourse import bacc
    from concourse.masks import make_identity

    dt = mybir.dt
    MM_DT = getattr(dt, mm_dt_name)
    f32 = dt.float32
    i32 = dt.int32
    Alu = mybir.AluOpType
    Act = mybir.ActivationFunctionType
    NSLOT = len(caps)
    CMAX = max(caps)

    def mm(ap):
        return ap

    nc = bacc.Bacc(None, target_bir_lowering=False, debug=False, num_devices=NCORES)

    # ---------------- I/O ----------------
    xfull = nc.dram_tensor("xfull", [T, H], f32, kind="ExternalInput")
    xsh = nc.dram_tensor("xsh", [TSH, H], f32, kind="ExternalInput")
    Wr1 = nc.dram_tensor("Wr1", [H, H], f32, kind="ExternalInput")
    br1 = nc.dram_tensor("br1", [H], f32, kind="ExternalInput")
    Wr2 = nc.dram_tensor("Wr2", [H, E], f32, kind="ExternalInput")
    br2 = nc.dram_tensor("br2", [E], f32, kind="ExternalInput")
    W1loc = nc.dram_tensor("W1loc", [NSLOT, H, F], MM_DT, kind="ExternalInput")
    b1loc = nc.dram_tensor("b1loc", [NSLOT, F], f32, kind="ExternalInput")
    W2loc = nc.dram_tensor("W2loc", [NSLOT, F, H], MM_DT, kind="ExternalInput")
    b2loc = nc.dram_tensor("b2loc", [NSLOT, H], MM_DT, kind="ExternalInput")
    ohloc = nc.dram_tensor("ohloc", [NSLOT, E], f32, kind="ExternalInput")
    slotlo = nc.dram_tensor("slotlo", [NSLOT], f32, kind="ExternalInput")
    out_sh = nc.dram_tensor("out_sh", [TSH, H], f32, kind="ExternalOutput")

    # ---------------- constants (inline in NEFF) ----------------
    u128 = nc.inline_tensor(np.triu(np.ones((128, 128), np.float32), 1), "u128")
    u16 = nc.inline_tensor(np.triu(np.ones((16, 16), np.float32), 1), "u16")
    ones128 = nc.inline_tensor(np.ones((128, 1), np.float32), "ones128")
    tokid_np = (np.arange(16)[None, :] * 128 + np.arange(128)[:, None]).astype(np.int32)
    tokid = nc.inline_tensor(tokid_np, "tokid")
    idxinit = nc.inline_tensor(np.full((CMAX + 1, 1), DUMP, np.int32), "idxinit")

    # ---------------- internal DRAM ----------------
    c2aug = nc.dram_tensor("c2aug", [T + 1, NSLOT], f32)
    idxb = [nc.dram_tensor(f"idxb{k}", [caps[k] + 1, 1], i32) for k in range(NSLOT)]
    outp2 = [nc.dram_tensor(f"outp{h}", [T + 1, H // 2], f32) for h in range(2)]
    agin = nc.dram_tensor("agin", [TSH, E], f32)
    call = nc.dram_tensor("call", [T, E], f32, addr_space="Shared")
    rsout2 = [nc.dram_tensor(f"rsout{h}", [TSH, H // 2], f32) for h in range(2)]

    RG = [list(range(NCORES))]

    with tile.TileContext(nc) as tc:
        with (
            tc.tile_pool(name="const", bufs=1) as constp,
            tc.tile_pool(name="persist", bufs=1) as persist,
        ):
            ident = constp.tile([128, 128], f32)
            make_identity(nc, ident)
            u128_sb = constp.tile_from(u128.ap())
            u16_sb = constp.tile_from(u16.ap())
            ones128_sb = constp.tile_from(ones128.ap())
            tokid_sb = constp.tile_from(tokid.ap())
            onesmm_f32 = constp.tile([1, 128], f32)
            nc.vector.memset(onesmm_f32[:], 1.0)
            onesmm_sb = constp.tile([1, 128], MM_DT)
            nc.vector.tensor_copy(onesmm_sb[:], onesmm_f32[:])
            zero_sb = constp.tile([128, H], f32)
            nc.vector.memset(zero_sb[:], 0.0)

            # ====== phase 1: router on this core's 256-token shard (fp32) ======
            with (
                tc.tile_pool(name="rweights", bufs=1) as rw,
                tc.tile_pool(name="rtmp", bufs=3) as rt,
                tc.tile_pool(name="rpsum", bufs=2, space="PSUM") as rp,
                tc.tile_pool(name="rtpsum", bufs=2, space="PSUM") as rtp,
            ):
                xt_sh = persist.tile([128, H // 128, TSH], f32)
                for t2 in range(TSH // 128):
                    xs = rt.tile([128, H], f32, tag="xs")
                    nc.sync.dma_start(xs[:], xsh[t2 * 128:(t2 + 1) * 128, :])
                    for hcc in range(H // 128):
                        tp = rtp.tile([128, 128], f32, tag="tp")
                        nc.tensor.transpose(tp[:], xs[:, hcc * 128:(hcc + 1) * 128], ident[:])
                        nc.any.tensor_copy(xt_sh[:, hcc, t2 * 128:(t2 + 1) * 128], tp[:])

                wr1_sb = rw.tile([128, H // 128, H], f32)
                nc.sync.dma_start(wr1_sb[:], Wr1.ap().rearrange("(c p) o -> p c o", p=128))
                wr2_sb = rw.tile([128, H // 128, E], f32)
                nc.sync.dma_start(wr2_sb[:], Wr2.ap().rearrange("(c p) e -> p c e", p=128))
                br1_sb = rw.tile([128, H // 128], f32)
                nc.sync.dma_start(br1_sb[:], br1.ap().rearrange("(c p) -> p c", p=128))
                br2_rep = rw.tile([128, E], f32)
                nc.sync.dma_start(
                    br2_rep[:],
                    br2.ap().rearrange("(o e) -> o e", o=1).to_broadcast([128, E]))

                r1t = persist.tile([128, H // 128, TSH], f32)
                for ho in range(H // 128):
                    p1 = rp.tile([128, TSH], f32, tag="p1")
                    for hc in range(H // 128):
                        nc.tensor.matmul(
                            p1[:], wr1_sb[:, hc, ho * 128:(ho + 1) * 128], xt_sh[:, hc, :],
                            start=(hc == 0), stop=(hc == H // 128 - 1))
                    nc.scalar.activation(r1t[:, ho, :], p1[:], Act.Relu,
                                         bias=br1_sb[:, ho:ho + 1])

                for t2 in range(TSH // 128):
                    p2 = rp.tile([128, E], f32, tag="p2")
                    for hc in range(H // 128):
                        nc.tensor.matmul(
                            p2[:], r1t[:, hc, t2 * 128:(t2 + 1) * 128], wr2_sb[:, hc, :],
                            start=(hc == 0), stop=(hc == H // 128 - 1))
                    lg = rt.tile([128, E], f32, tag="lg")
                    nc.vector.tensor_tensor(lg[:], p2[:], br2_rep[:], op=Alu.add)
                    mx8 = rt.tile([128, 8], f32, tag="mx8")
                    nc.vector.max(mx8[:], lg[:])
                    mx4 = rt.tile([128, 8], f32, tag="mx4")
                    nc.vector.memset(mx4[:], NEG)
                    nc.vector.tensor_copy(mx4[:, 0:TOPK], mx8[:, 0:TOPK])
                    zap = rt.tile([128, E], f32, tag="zap")
                    nc.vector.match_replace(zap[:], in_to_replace=mx4[:], in_values=lg[:],
                                            imm_value=NEG)
                    mask = rt.tile([128, E], f32, tag="mask")
                    nc.vector.tensor_tensor(mask[:], lg[:], zap[:], op=Alu.not_equal)
                    negmax = rt.tile([128, 1], f32, tag="negmax")
                    nc.vector.tensor_scalar_mul(negmax[:], mx8[:, 0:1], -1.0)
                    ex = rt.tile([128, E], f32, tag="ex")
                    nc.scalar.activation(ex[:], lg[:], Act.Exp, bias=negmax[:])
                    nc.vector.tensor_tensor(ex[:], ex[:], mask[:], op=Alu.mult)
                    den = rt.tile([128, 1], f32, tag="den")
                    nc.vector.reduce_sum(den[:], ex[:], axis=mybir.AxisListType.X)
                    rcp = rt.tile([128, 1], f32, tag="rcp")
                    nc.vector.reciprocal(rcp[:], den[:])
                    csh = rt.tile([128, E], f32, tag="csh")
                    nc.vector.tensor_scalar(csh[:], ex[:], rcp[:], None, op0=Alu.mult)
                    nc.sync.dma_start(agin[t2 * 128:(t2 + 1) * 128, :], csh[:])

            nc.gpsimd.collective_compute(
                "AllGather", Alu.bypass, replica_groups=RG,
                ins=[agin.ap().opt()], outs=[call.ap().opt()])

            # deferred init (sync queue is idle while the router computes):
            # zero accumulator, init idx buffers, zero c2aug dump row
            for k in range(NSLOT):
                nc.sync.dma_start(idxb[k][:], idxinit.ap()[0:caps[k] + 1, :])
            nc.sync.dma_start(c2aug[T:T + 1, :], zero_sb[0:1, 0:NSLOT])
            for h in range(2):
                for k in range(T // 128):
                    nc.sync.dma_start(outp2[h][k * 128:(k + 1) * 128, :],
                                      zero_sb[:, 0:H // 2])
                nc.sync.dma_start(outp2[h][T:T + 1, :], zero_sb[0:1, 0:H // 2])

            # ====== phase 2: dispatch for the NSLOT local slots ======
            idx_sb = []
            s_col = []
            with (
                tc.tile_pool(name="dsb", bufs=2) as dsb,
                tc.tile_pool(name="dps", bufs=2, space="PSUM") as dps,
            ):
                cf = persist.tile([128, T // 128, E], f32, tag="cfall")
                nc.sync.dma_start(cf[:], call.ap().rearrange("(c p) e -> p c e", p=128))
                ohrep = dsb.tile([128, NSLOT, E], f32, tag="ohrep")
                nc.sync.dma_start(
                    ohrep[:],
                    ohloc.ap().rearrange("(o l) e -> o l e", o=1).to_broadcast([128, NSLOT, E]))
                lo_rep = dsb.tile([128, NSLOT], f32, tag="lo_rep")
                nc.sync.dma_start(
                    lo_rep[:],
                    slotlo.ap().rearrange("(o l) -> o l", o=1).to_broadcast([128, NSLOT]))

                c2sb = persist.tile([128, T // 128, NSLOT], f32)
                xg0_hold = [persist.tile([128, H], f32, tag=f"xg0_{_i}", name=f"xg0_{_i}")
                            for _i in range(caps[0] // 128)]
                for k in range(NSLOT):
                    idx_sb.append(persist.tile([128, caps[k] // 128], i32,
                                               tag=f"idx{k}", name=f"idx{k}"))
                    s_col.append(persist.tile([128, caps[k] // 128], f32,
                                              tag=f"scol{k}", name=f"scol{k}"))

                for k in range(NSLOT):
                    A = caps[k]
                    msk = dsb.tile([128, T // 128, E], f32, tag="msk")
                    nc.vector.tensor_tensor(
                        msk[:], cf[:],
                        ohrep[:, k:k + 1, :].to_broadcast([128, T // 128, E]),
                        op=Alu.mult)
                    ce = dsb.tile([128, T // 128], f32, tag="ce")
                    nc.vector.reduce_sum(ce[:], msk[:], axis=mybir.AxisListType.X)
                    nc.vector.tensor_copy(c2sb[:, :, k], ce[:])
                    m = dsb.tile([128, T // 128], f32, tag="m")
                    nc.vector.tensor_scalar(m[:], ce[:], 0.0, None, op0=Alu.not_equal)

                    # exclusive cumsum over global token order (partition-inner)
                    csp = dps.tile([16, 1], f32, tag="csp")
                    nc.tensor.matmul(csp[:], m[:], ones128_sb[:], start=True, stop=True)
                    cs_sb = dsb.tile([16, 1], f32, tag="cs_sb")
                    nc.any.tensor_copy(cs_sb[:], csp[:])
                    csrep = dsb.tile([16, 128], f32, tag="csrep")
                    nc.vector.tensor_copy(csrep[:], cs_sb[:].to_broadcast([16, 128]))
                    posp = dps.tile([128, T // 128], f32, tag="posp")
                    nc.tensor.matmul(posp[:], u128_sb[:], m[:], start=True, stop=False)
                    nc.tensor.matmul(posp[:], csrep[:], u16_sb[:], start=False, stop=True)

                    # gate to [lo, lo+A): tpos = pos - lo; m' = m*(tpos>=0)*(tpos<A)
                    tpos = dsb.tile([128, T // 128], f32, tag="tpos")
                    nc.vector.tensor_scalar(tpos[:], posp[:], lo_rep[:, k:k + 1], None,
                                            op0=Alu.subtract)
                    g1 = dsb.tile([128, T // 128], f32, tag="g1")
                    nc.vector.tensor_scalar(g1[:], tpos[:], 0.0, None, op0=Alu.is_ge)
                    g2 = dsb.tile([128, T // 128], f32, tag="g2")
                    nc.vector.tensor_scalar(g2[:], tpos[:], float(A), None, op0=Alu.is_lt)
                    nc.vector.tensor_tensor(m[:], m[:], g1[:], op=Alu.mult)
                    nc.vector.tensor_tensor(m[:], m[:], g2[:], op=Alu.mult)

                    # offsets: O = A + m*(tpos - A)   (unselected -> dump slot A)
                    of = dsb.tile([128, T // 128], f32, tag="of")
                    nc.vector.tensor_scalar(of[:], tpos[:], float(A), None, op0=Alu.subtract)
                    nc.vector.tensor_tensor(of[:], of[:], m[:], op=Alu.mult)
                    nc.vector.tensor_scalar(of[:], of[:], float(A), None, op0=Alu.add)
                    oi = dsb.tile([128, T // 128], i32, tag="oi")
                    nc.vector.tensor_copy(oi[:], of[:])

                    for c in range(T // 128):
                        nc.gpsimd.indirect_dma_start(
                            out=idxb[k].ap(),
                            out_offset=bass.IndirectOffsetOnAxis(ap=oi[:, c:c + 1], axis=0),
                            in_=tokid_sb[:, c:c + 1], in_offset=None,
                            bounds_check=A, oob_is_err=False)

                    nc.sync.dma_start(
                        idx_sb[k][:],
                        idxb[k].ap()[0:A, :].rearrange("(c p) o -> p (c o)", p=128))

                for k in range(T // 128):
                    nc.sync.dma_start(c2aug[k * 128:(k + 1) * 128, :], c2sb[:, k, :])
                for k in range(NSLOT):
                    if k == 0:
                        # hoist slot-0 row gathers ahead of everything else on
                        # the Pool queue so its transposes/mm1 start earliest
                        for ck in range(caps[0] // 128):
                            xg = xg0_hold[ck]
                            nc.gpsimd.indirect_dma_start(
                                out=xg[:], out_offset=None,
                                in_=xfull.ap(),
                                in_offset=bass.IndirectOffsetOnAxis(
                                    ap=idx_sb[0][:, ck:ck + 1], axis=0),
                                bounds_check=T - 1, oob_is_err=False)
                    for j in range(caps[k] // 128):
                        nc.gpsimd.indirect_dma_start(
                            out=s_col[k][:, j:j + 1], out_offset=None,
                            in_=c2aug.ap(),
                            in_offset=bass.IndirectOffsetOnAxis(
                                ap=idx_sb[k][:, j:j + 1], axis=0),
                            element_offset=k,
                            bounds_check=T, oob_is_err=True)

            # ====== phase 3: expert MLP per slot ======
            FO = 512
            HH = H // 2
            with (
                tc.tile_pool(name="xg", bufs=2) as xgp,
                tc.tile_pool(name="w1", bufs=16) as w1p,
                tc.tile_pool(name="w2", bufs=4) as w2p,
                tc.tile_pool(name="hbuf", bufs=1) as hbp,
                tc.tile_pool(name="xt", bufs=1) as xtp,
                tc.tile_pool(name="ysb", bufs=4) as ysp,
                tc.tile_pool(name="bias", bufs=1) as biasp,
                tc.tile_pool(name="psh", bufs=2, space="PSUM") as psh,
                tc.tile_pool(name="psy", bufs=5, space="PSUM") as psy,
                tc.tile_pool(name="pst", bufs=1, space="PSUM") as pst,
            ):
                b1_sb = biasp.tile([128, NSLOT, F // 128], f32)
                nc.sync.dma_start(b1_sb[:], b1loc.ap().rearrange("l (c p) -> p l c", p=128))

                for k in range(NSLOT):
                    A = caps[k]
                    b2_sb = biasp.tile([1, H], MM_DT, tag="b2_sb")
                    nc.sync.dma_start(
                        b2_sb[:], b2loc.ap()[k:k + 1, :])
                    chs = [A] if A <= 512 else [A // 2, A // 2]

                    xt = xtp.tile([128, H // 128, CMAX], MM_DT, tag="xt")
                    for ck in range(A // 128):
                        if k == 0:
                            xg = xg0_hold[ck]
                        else:
                            xg = xgp.tile([128, H], f32, tag="xg")
                            nc.gpsimd.indirect_dma_start(
                                out=xg[:], out_offset=None,
                                in_=xfull.ap(),
                                in_offset=bass.IndirectOffsetOnAxis(
                                    ap=idx_sb[k][:, ck:ck + 1], axis=0),
                                bounds_check=T - 1, oob_is_err=False)
                        for hc in range(H // 128):
                            tp = pst.tile([128, 128], f32, tag="tp3")
                            nc.tensor.transpose(tp[:], xg[:, hc * 128:(hc + 1) * 128], ident[:])
                            nc.any.tensor_copy(xt[:, hc, ck * 128:(ck + 1) * 128], tp[:])

                    # mm1: h^T[f, c] = gelu(sum_h W1[h,f]^T x^T[h,c] + b1[f])
                    hbuf = hbp.tile([128, F // 128, CMAX], MM_DT, tag="hbuf")
                    for fo in range(F // FO):
                        w1sb = [w1p.tile([128, FO], MM_DT, tag="w1sb", name=f"w1sb{_i}")
                                for _i in range(H // 128)]
                        for hc in range(H // 128):
                            nc.sync.dma_start(
                                w1sb[hc][:],
                                W1loc[k, hc * 128:(hc + 1) * 128, fo * FO:(fo + 1) * FO])
                        for fi in range(FO // 128):
                            fg = fo * (FO // 128) + fi
                            cc0 = 0
                            for ch in chs:
                                ph = psh.tile([128, 512], f32, tag="ph")
                                for hc in range(H // 128):
                                    nc.tensor.matmul(
                                        ph[:, 0:ch], mm(w1sb[hc][:, fi * 128:(fi + 1) * 128]),
                                        mm(xt[:, hc, cc0:cc0 + ch]),
                                        start=(hc == 0), stop=(hc == H // 128 - 1))
                                nc.scalar.activation(
                                    hbuf[:, fg, cc0:cc0 + ch], ph[:, 0:ch],
                                    Act.Gelu, bias=b1_sb[:, k, fg:fg + 1])
                                cc0 += ch

                    # mm2: y[c, h] = (sum_f h^T[f,c]^T W2[f,h] + b2[h]) * s[c]
                    for hh in range(H // HH):
                        pys = [psy.tile([128, HH], f32, tag="py", name=f"py{_i}")
                               for _i in range(A // 128)]
                        for fg in range(F // 128):
                            w2sb = w2p.tile([128, HH], MM_DT, tag="w2sb")
                            nc.sync.dma_start(
                                w2sb[:],
                                W2loc[k, fg * 128:(fg + 1) * 128, hh * HH:(hh + 1) * HH])
                            for ck in range(A // 128):
                                nc.tensor.matmul(
                                    pys[ck][:], mm(hbuf[:, fg, ck * 128:(ck + 1) * 128]),
                                    mm(w2sb[:]), start=(fg == 0), stop=False)
                        for ck in range(A // 128):
                            nc.tensor.matmul(
                                pys[ck][:], mm(onesmm_sb[0:1, :]),
                                mm(b2_sb[0:1, hh * HH:(hh + 1) * HH]),
                                start=False, stop=True)
                            ysb = ysp.tile([128, HH], f32, tag="ysb")
                            nc.vector.tensor_scalar(
                                ysb[:], pys[ck][:], s_col[k][:, ck:ck + 1], None,
                                op0=Alu.mult)
                            nc.gpsimd.indirect_dma_start(
                                out=outp2[hh].ap(),
                                out_offset=bass.IndirectOffsetOnAxis(
                                    ap=idx_sb[k][:, ck:ck + 1], axis=0),
                                in_=ysb[:], in_offset=None,
                                compute_op=Alu.add,
                                bounds_check=T, oob_is_err=True)
                        if k == NSLOT - 1 and hh == 0 and ck == A // 128 - 1:
                            # all h-half-0 contributions are in: start its
                            # ReduceScatter now so it overlaps h-half-1 compute
                            nc.gpsimd.collective_compute(
                                "ReduceScatter", Alu.add, replica_groups=RG,
                                ins=[outp2[0].ap()[0:T, :].opt()],
                                outs=[rsout2[0].ap().opt()])

            # ====== phase 4: remaining reduce + output shard ======
            # (h-half 0's ReduceScatter was emitted inline above)
            with tc.tile_pool(name="outc", bufs=2) as outc:
                for k in range(TSH // 128):
                    ot = outc.tile([128, H // 2], f32, tag="ot")
                    nc.sync.dma_start(ot[:], rsout2[0][k * 128:(k + 1) * 128, :])
                    nc.sync.dma_start(out_sh[k * 128:(k + 1) * 128, 0:H // 2], ot[:])
                nc.gpsimd.collective_compute(
                    "ReduceScatter", Alu.add, replica_groups=RG,
                    ins=[outp2[1].ap()[0:T, :].opt()], outs=[rsout2[1].ap().opt()])
                for k in range(TSH // 128):
                    ot = outc.tile([128, H // 2], f32, tag="ot")
                    nc.sync.dma_start(ot[:], rsout2[1][k * 128:(k + 1) * 128, :])
                    nc.sync.dma_start(
                        out_sh[k * 128:(k + 1) * 128, H // 2:H], ot[:])

    nc.compile()
    if not nc.is_finalized():
        nc.finalize()
    return nc


def _in_maps(inputs, caps, asg):
    NSLOT = len(caps)
    x = np.ascontiguousarray(np.asarray(inputs["x"], np.float32).reshape(T, H))
    W1 = np.asarray(inputs["W1"], np.float32)
    b1 = np.asarray(inputs["b1"], np.float32)
    W2 = np.asarray(inputs["W2"], np.float32)
    b2 = np.asarray(inputs["b2"], np.float32)
    common = {
        "xfull": x,
        "Wr1": np.ascontiguousarray(np.asarray(inputs["Wr1"], np.float32)),
        "br1": np.ascontiguousarray(np.asarray(inputs["br1"], np.float32)),
        "Wr2": np.ascontiguousarray(np.asarray(inputs["Wr2"], np.float32)),
        "br2": np.ascontiguousarray(np.asarray(inputs["br2"], np.float32)),
    }
    maps = []
    for r in range(NCORES):
        w1l = np.empty((NSLOT, H, F), np.float32)
        b1l = np.empty((NSLOT, F), np.float32)
        w2l = np.empty((NSLOT, F, H), np.float32)
        b2l = np.empty((NSLOT, H), np.float32)
        oh = np.zeros((NSLOT, E), np.float32)
        lo = np.zeros((NSLOT,), np.float32)
        for kk in range(NSLOT):
            e, l0 = asg[kk][r]
            w1l[kk] = W1[e]
            b1l[kk] = b1[e]
            w2l[kk] = W2[e]
            b2l[kk] = b2[e]
            if l0 <= T:
                oh[kk, e] = 1.0     # empty slots keep an all-zero one-hot
            lo[kk] = float(l0)
        maps.append({
            **common,
            "xsh": np.ascontiguousarray(x[r * TSH:(r + 1) * TSH]),
            "W1loc": w1l, "b1loc": b1l, "W2loc": w2l, "b2loc": b2l,
            "ohloc": oh, "slotlo": lo,
        })
    return maps


def _get_nc(caps, mm_dt_name="float32r"):
    key = (tuple(caps), mm_dt_name)
    if key not in _CACHE:
        _CACHE[key] = _build(list(caps), mm_dt_name)
    return _CACHE[key]


def kernel(**inputs) -> np.ndarray:
    from concourse.bass_utils import run_bass_kernel_spmd

    caps, asg = _plan(inputs)
    nc = _get_nc(caps)
    maps = _in_maps(inputs, caps, asg)
    res = run_bass_kernel_spmd(nc, maps, core_ids=list(range(NCORES)))
    shards = [res.results[r]["out_sh"] for r in range(NCORES)]
    out = np.concatenate(shards, axis=0).reshape(np.asarray(inputs["x"]).shape)
    return out.astype(np.float32)



# revision 8
# speedup vs baseline: 2.5136x; 2.5136x over previous
# kernel.py — MoE (E=16, top-4) Trainium2 Bass kernel, expert-parallel over 8 cores.
#
# v2 strategy (from v1 sim/HW analysis: DMA-saturated fp32 weight stream,
# 75us dispatch dead zone before the first expert matmul):
#   - Router (Linear->ReLU->Linear, top-4 softmax) computed data-parallel,
#     f32r matmuls for the HxH layer; AllGather of the dense combine weights.
#   - Host-planned (expert, pos-range) pieces packed into NSLOT=3 slots/core
#     (128-granular capacities); device recomputes routing exactly and
#     dispatches itself.
#   - Dispatch per slot: mask -> exclusive cumsum (PE) -> gate to [lo,lo+cap)
#     -> indirect-DMA scatter of (tokid, combine) float pairs -> load back ->
#     bf16 gather of routed token rows. No separate combine-weight gather.
#   - Expert MLP in bf16 (weights host-cast): h^T = gelu(W1^T x^T + b1),
#     y = (h W2 + b2) * combine; fp32 scatter-add into [T,H] accumulator.
#   - Weights stream on the ACT (scalar) HWDGE queue in 1-4MB chunks;
#     sync queue keeps router loads / inits / small loads.
#   - ReduceScatter(add) over 8 cores in two H-halves (first overlaps the
#     last slot's second-half compute); each core emits its 256-token shard.
import numpy as np

H = 1024
F = 4096
E = 16
TOPK = 4
T = 2048
NCORES = 8
TSH = T // NCORES          # 256 router tokens per core
NEG = -3.0e38
MARGIN = 32                # slack over host-computed counts (host/device drift)
NSLOT = 3
CAP_OPTS = [128, 256, 384, 512, 640]

_CACHE = {}


# ---------------------------------------------------------------------------
# Host-side planning: counts -> slot capacities + (expert, lo) assignment
# ---------------------------------------------------------------------------

def _host_counts(inputs):
    x = np.asarray(inputs["x"], np.float32).reshape(T, H)
    h = np.maximum(x @ np.asarray(inputs["Wr1"], np.float32)
                   + np.asarray(inputs["br1"], np.float32), 0.0)
    lg = h @ np.asarray(inputs["Wr2"], np.float32) + np.asarray(inputs["br2"], np.float32)
    order = np.argsort(-lg, axis=1, kind="stable")[:, :TOPK]
    counts = np.zeros(E, np.int64)
    for e in range(E):
        counts[e] = (order == e).sum()
    return counts


def _split_pieces(need, avail, sizes):
    """Split each expert's need into pieces drawn from avail (cap -> count).
    Best-fit: smallest single cap that covers the tail; else combinations
    that avoid burning large caps on small tails."""
    pieces = []
    for e in np.argsort(-need):
        rem = int(need[e])
        lo = 0
        while rem > 0:
            pick = None
            # smallest available cap that covers the remainder
            for a in sorted(sizes):
                if avail.get(a, 0) > 0 and a >= rem:
                    pick = a
                    break
            if pick is not None:
                # check whether two smaller caps cover it with less waste
                best_pair = None
                for a in sorted(sizes):
                    if a >= pick or avail.get(a, 0) == 0:
                        continue
                    need2 = rem - a
                    for b in sorted(sizes):
                        if b >= pick or avail.get(b, 0) == 0 or b < need2:
                            continue
                        if a == b and avail.get(a, 0) < 2:
                            continue
                        if best_pair is None or a + b < best_pair[0] + best_pair[1]:
                            best_pair = (a, b)
                        break
                if best_pair is not None and sum(best_pair) < pick:
                    a, b = best_pair
                    avail[a] -= 1
                    pieces.append((a, int(e), lo))
                    lo += a
                    rem -= a
                    continue
            if pick is None:           # largest available, keep going
                for a in sorted(sizes, reverse=True):
                    if avail.get(a, 0) > 0:
                        pick = a
                        break
            if pick is None:
                return None
            avail[pick] -= 1
            pieces.append((pick, int(e), lo))
            lo += pick
            rem -= pick
    return pieces


def _try_pack(need, caps):
    avail = {}
    for a in caps:
        avail[a] = avail.get(a, 0) + NCORES
    pieces = _split_pieces(need, dict(avail), sorted(set(caps)))
    if pieces is None:
        return None
    slots = {a: [] for a in set(caps)}
    for cap, e, lo in pieces:
        slots[cap].append((e, lo))
    out = []
    used = {a: 0 for a in set(caps)}
    for a in caps:
        pos = []
        for r in range(NCORES):
            i = used[a]
            if i < len(slots[a]):
                pos.append(slots[a][i])
            else:
                pos.append((0, T + 4096))   # empty slot: range never matches
            used[a] += 1
        out.append(pos)
    return out


def _plan(inputs):
    counts = _host_counts(inputs)
    need = counts + MARGIN
    cands = []
    for a in CAP_OPTS:
        for b in CAP_OPTS:
            if b > a:
                continue
            for c in CAP_OPTS:
                if c > b:
                    continue
                if NCORES * (a + b + c) >= int(need.sum()):
                    cands.append((a, b, c))
    cands.sort(key=lambda t: (sum(t), t[0]))
    for caps in cands:
        asg = _try_pack(need, list(caps))
        if asg is not None:
            return list(caps), asg
    raise RuntimeError(f"no feasible slot packing for counts {counts}")


# ---------------------------------------------------------------------------
# Device program
# ---------------------------------------------------------------------------

def _build(caps):
    import concourse.bass as bass
    import concourse.mybir as mybir
    import concourse.tile as tile
    from concourse import bacc
    from concourse.masks import make_identity

    dt = mybir.dt
    bf16 = dt.bfloat16
    f32 = dt.float32
    i32 = dt.int32
    Alu = mybir.AluOpType
    Act = mybir.ActivationFunctionType
    CMAX = max(caps)

    nc = bacc.Bacc(None, target_bir_lowering=False, debug=False, num_devices=NCORES)

    # ---------------- I/O ----------------
    xbf = nc.dram_tensor("xbf", [T, H], bf16, kind="ExternalInput")
    xsh = nc.dram_tensor("xsh", [TSH, H], f32, kind="ExternalInput")
    Wr1 = nc.dram_tensor("Wr1", [H, H], f32, kind="ExternalInput")
    br1 = nc.dram_tensor("br1", [H], f32, kind="ExternalInput")
    Wr2 = nc.dram_tensor("Wr2", [H, E], f32, kind="ExternalInput")
    br2 = nc.dram_tensor("br2", [E], f32, kind="ExternalInput")
    W1loc = nc.dram_tensor("W1loc", [NSLOT, H, F], bf16, kind="ExternalInput")
    b1loc = nc.dram_tensor("b1loc", [NSLOT, F], f32, kind="ExternalInput")
    W2loc = nc.dram_tensor("W2loc", [NSLOT, F, H], bf16, kind="ExternalInput")
    b2loc = nc.dram_tensor("b2loc", [NSLOT, H], bf16, kind="ExternalInput")
    ohloc = nc.dram_tensor("ohloc", [NSLOT, E], f32, kind="ExternalInput")
    slotlo = nc.dram_tensor("slotlo", [NSLOT], f32, kind="ExternalInput")
    out_sh = nc.dram_tensor("out_sh", [TSH, H], f32, kind="ExternalOutput")

    # ---------------- constants (inline in NEFF) ----------------
    u128 = nc.inline_tensor(np.triu(np.ones((128, 128), np.float32), 1), "u128")
    u16 = nc.inline_tensor(np.triu(np.ones((16, 16), np.float32), 1), "u16")
    ones128 = nc.inline_tensor(np.ones((128, 1), np.float32), "ones128")
    tokid_np = (np.arange(16)[None, :] * 128 + np.arange(128)[:, None]).astype(np.float32)
    tokidf = nc.inline_tensor(tokid_np, "tokidf")
    zeros2 = nc.inline_tensor(np.zeros((CMAX + 1, 2), np.float32), "zeros2")

    # ---------------- internal DRAM ----------------
    # (tokid, combine) pairs per slot position; row cap[k] is the dump row.
    idxcb = [nc.dram_tensor(f"idxcb{k}", [caps[k] + 1, 2], f32) for k in range(NSLOT)]
    outp2 = [nc.dram_tensor(f"outp{h}", [T + 1, H // 2], f32) for h in range(2)]
    agin = nc.dram_tensor("agin", [TSH, E], f32)
    call = nc.dram_tensor("call", [T, E], f32, addr_space="Shared")
    rsout2 = [nc.dram_tensor(f"rsout{h}", [TSH, H // 2], f32) for h in range(2)]

    RG = [list(range(NCORES))]

    with tile.TileContext(nc) as tc:
        with (
            tc.tile_pool(name="const", bufs=1) as constp,
            tc.tile_pool(name="persist", bufs=1) as persist,
        ):
            ident = constp.tile([128, 128], f32)
            make_identity(nc, ident)
            ident_bf = constp.tile([128, 128], bf16)
            nc.vector.tensor_copy(ident_bf[:], ident[:])
            u128_sb = constp.tile_from(u128.ap())
            u16_sb = constp.tile_from(u16.ap())
            ones128_sb = constp.tile_from(ones128.ap())
            tokidf_sb = constp.tile_from(tokidf.ap())
            onesmm_f32 = constp.tile([1, 128], f32)
            nc.vector.memset(onesmm_f32[:], 1.0)
            onesmm_sb = constp.tile([1, 128], bf16)
            nc.vector.tensor_copy(onesmm_sb[:], onesmm_f32[:])
            zero_sb = constp.tile([128, H // 2], f32)
            nc.vector.memset(zero_sb[:], 0.0)

            # ====== phase 1: router on this core's 256-token shard ======
            with (
                tc.tile_pool(name="rweights", bufs=1) as rw,
                tc.tile_pool(name="rtmp", bufs=3) as rt,
                tc.tile_pool(name="rbig", bufs=1) as rbig,
                tc.tile_pool(name="rpsum", bufs=2, space="PSUM") as rp,
                tc.tile_pool(name="rtpsum", bufs=2, space="PSUM") as rtp,
            ):
                xt_sh = rbig.tile([128, H // 128, TSH], f32)
                for t2 in range(TSH // 128):
                    xs = rt.tile([128, H], f32, tag="xs")
                    nc.sync.dma_start(xs[:], xsh[t2 * 128:(t2 + 1) * 128, :])
                    for hcc in range(H // 128):
                        tp = rtp.tile([128, 128], f32, tag="tp")
                        nc.tensor.transpose(tp[:], xs[:, hcc * 128:(hcc + 1) * 128], ident[:])
                        nc.vector.tensor_copy(xt_sh[:, hcc, t2 * 128:(t2 + 1) * 128], tp[:])

                wr1_sb = rw.tile([128, H // 128, H], f32)
                nc.sync.dma_start(wr1_sb[:], Wr1.ap().rearrange("(c p) o -> p c o", p=128))
                wr2_sb = rw.tile([128, H // 128, E], f32)
                nc.sync.dma_start(wr2_sb[:], Wr2.ap().rearrange("(c p) e -> p c e", p=128))
                br1_sb = rw.tile([128, H // 128], f32)
                nc.sync.dma_start(br1_sb[:], br1.ap().rearrange("(c p) -> p c", p=128))
                br2_rep = rw.tile([128, E], f32)
                nc.sync.dma_start(
                    br2_rep[:],
                    br2.ap().rearrange("(o e) -> o e", o=1).to_broadcast([128, E]))

                r1t = rbig.tile([128, H // 128, TSH], f32)
                for ho in range(H // 128):
                    p1 = rp.tile([128, TSH], f32, tag="p1")
                    for hc in range(H // 128):
                        nc.tensor.matmul(
                            p1[:], wr1_sb[:, hc, ho * 128:(ho + 1) * 128], xt_sh[:, hc, :],
                            start=(hc == 0), stop=(hc == H // 128 - 1))
                    nc.scalar.activation(r1t[:, ho, :], p1[:], Act.Relu,
                                         bias=br1_sb[:, ho:ho + 1])

                for t2 in range(TSH // 128):
                    p2 = rp.tile([128, E], f32, tag="p2")
                    for hc in range(H // 128):
                        nc.tensor.matmul(
                            p2[:], r1t[:, hc, t2 * 128:(t2 + 1) * 128], wr2_sb[:, hc, :],
                            start=(hc == 0), stop=(hc == H // 128 - 1))
                    lg = rt.tile([128, E], f32, tag="lg")
                    nc.vector.tensor_tensor(lg[:], p2[:], br2_rep[:], op=Alu.add)
                    mx8 = rt.tile([128, 8], f32, tag="mx8")
                    nc.vector.max(mx8[:], lg[:])
                    mx4 = rt.tile([128, 8], f32, tag="mx4")
                    nc.vector.memset(mx4[:], NEG)
                    nc.vector.tensor_copy(mx4[:, 0:TOPK], mx8[:, 0:TOPK])
                    zap = rt.tile([128, E], f32, tag="zap")
                    nc.vector.match_replace(zap[:], in_to_replace=mx4[:], in_values=lg[:],
                                            imm_value=NEG)
                    mask = rt.tile([128, E], f32, tag="mask")
                    nc.vector.tensor_tensor(mask[:], lg[:], zap[:], op=Alu.not_equal)
                    negmax = rt.tile([128, 1], f32, tag="negmax")
                    nc.vector.tensor_scalar_mul(negmax[:], mx8[:, 0:1], -1.0)
                    ex = rt.tile([128, E], f32, tag="ex")
                    nc.scalar.activation(ex[:], lg[:], Act.Exp, bias=negmax[:])
                    nc.vector.tensor_tensor(ex[:], ex[:], mask[:], op=Alu.mult)
                    den = rt.tile([128, 1], f32, tag="den")
                    nc.vector.reduce_sum(den[:], ex[:], axis=mybir.AxisListType.X)
                    rcp = rt.tile([128, 1], f32, tag="rcp")
                    nc.vector.reciprocal(rcp[:], den[:])
                    csh = rt.tile([128, E], f32, tag="csh")
                    nc.vector.tensor_scalar(csh[:], ex[:], rcp[:], None, op0=Alu.mult)
                    nc.sync.dma_start(agin[t2 * 128:(t2 + 1) * 128, :], csh[:])

            nc.gpsimd.collective_compute(
                "AllGather", Alu.bypass, replica_groups=RG,
                ins=[agin.ap().opt()], outs=[call.ap().opt()])

            # deferred init on the scalar queue (sync is busy with router
            # loads at t=0; the weight stream has slack for 8.4MB of zeros):
            # zero the (tokid, combine) buffers and the output accumulators
            for k in range(NSLOT):
                nc.scalar.dma_start(idxcb[k][:], zeros2.ap()[0:caps[k] + 1, :])
            for h in range(2):
                for k in range(T // 128):
                    nc.scalar.dma_start(outp2[h][k * 128:(k + 1) * 128, :], zero_sb[:])
                nc.scalar.dma_start(outp2[h][T:T + 1, :], zero_sb[0:1, :])

            # ====== phases 2+3: per-slot dispatch + expert MLP ======
            idx_sb = []
            s_col = []
            with (
                tc.tile_pool(name="dsb", bufs=2) as dsb,
                tc.tile_pool(name="xg", bufs=2) as xgp,
                tc.tile_pool(name="w1", bufs=3) as w1p,
                tc.tile_pool(name="w2", bufs=5) as w2p,
                tc.tile_pool(name="hbuf", bufs=2) as hbp,
                tc.tile_pool(name="xt", bufs=2) as xtp,
                tc.tile_pool(name="ysb", bufs=2) as ysp,
                tc.tile_pool(name="bias", bufs=1) as biasp,
                tc.tile_pool(name="psh", bufs=2, space="PSUM") as psh,
                tc.tile_pool(name="psy", bufs=2, space="PSUM") as psy,
                tc.tile_pool(name="pss", bufs=1, space="PSUM") as pss,
            ):
                cf = persist.tile([128, T // 128, E], f32, tag="cfall")
                nc.sync.dma_start(cf[:], call.ap().rearrange("(c p) e -> p c e", p=128))
                ohrep = dsb.tile([128, NSLOT, E], f32, tag="ohrep")
                nc.sync.dma_start(
                    ohrep[:],
                    ohloc.ap().rearrange("(o l) e -> o l e", o=1).to_broadcast([128, NSLOT, E]))
                lo_rep = dsb.tile([128, NSLOT], f32, tag="lo_rep")
                nc.sync.dma_start(
                    lo_rep[:],
                    slotlo.ap().rearrange("(o l) -> o l", o=1).to_broadcast([128, NSLOT]))
                b1_sb = biasp.tile([128, NSLOT, F // 128], f32)
                nc.sync.dma_start(b1_sb[:], b1loc.ap().rearrange("l (c p) -> p l c", p=128))

                for k in range(NSLOT):
                    A = caps[k]
                    NCK = A // 128

                    # ---- dispatch for slot k ----
                    msk = dsb.tile([128, T // 128, E], f32, tag="msk")
                    nc.vector.tensor_tensor(
                        msk[:], cf[:],
                        ohrep[:, k:k + 1, :].to_broadcast([128, T // 128, E]),
                        op=Alu.mult)
                    ce = dsb.tile([128, T // 128], f32, tag="ce")
                    nc.vector.reduce_sum(ce[:], msk[:], axis=mybir.AxisListType.X)
                    m = dsb.tile([128, T // 128], f32, tag="m")
                    nc.vector.tensor_scalar(m[:], ce[:], 0.0, None, op0=Alu.not_equal)

                    # exclusive cumsum over global token order (partition-inner)
                    csp = pss.tile([16, 1], f32, tag="csp")
                    nc.tensor.matmul(csp[:], m[:], ones128_sb[:], start=True, stop=True)
                    cs_sb = dsb.tile([16, 1], f32, tag="cs_sb")
                    nc.vector.tensor_copy(cs_sb[:], csp[:])
                    csrep = dsb.tile([16, 128], f32, tag="csrep")
                    nc.vector.tensor_copy(csrep[:], cs_sb[:].to_broadcast([16, 128]))
                    posp = pss.tile([128, T // 128], f32, tag="posp")
                    nc.tensor.matmul(posp[:], u128_sb[:], m[:], start=True, stop=False)
                    nc.tensor.matmul(posp[:], csrep[:], u16_sb[:], start=False, stop=True)

                    # gate to [lo, lo+A): tpos = pos - lo; m' = m*(tpos>=0)*(tpos<A)
                    tpos = dsb.tile([128, T // 128], f32, tag="tpos")
                    nc.vector.tensor_scalar(tpos[:], posp[:], lo_rep[:, k:k + 1], None,
                                            op0=Alu.subtract)
                    g1 = dsb.tile([128, T // 128], f32, tag="g1")
                    nc.vector.tensor_scalar(g1[:], tpos[:], 0.0, None, op0=Alu.is_ge)
                    g2 = dsb.tile([128, T // 128], f32, tag="g2")
                    nc.vector.tensor_scalar(g2[:], tpos[:], float(A), None, op0=Alu.is_lt)
                    nc.vector.tensor_tensor(m[:], m[:], g1[:], op=Alu.mult)
                    nc.vector.tensor_tensor(m[:], m[:], g2[:], op=Alu.mult)

                    # offsets: O = A + m*(tpos - A)   (unselected -> dump row A)
                    of = dsb.tile([128, T // 128], f32, tag="of")
                    nc.vector.tensor_scalar(of[:], tpos[:], float(A), None, op0=Alu.subtract)
                    nc.vector.tensor_tensor(of[:], of[:], m[:], op=Alu.mult)
                    nc.vector.tensor_scalar(of[:], of[:], float(A), None, op0=Alu.add)
                    oi = dsb.tile([128, T // 128], i32, tag="oi")
                    nc.vector.tensor_copy(oi[:], of[:])

                    # (tokid, combine) pair rows, scattered to slot positions
                    val2 = dsb.tile([128, T // 128, 2], f32, tag="val2")
                    nc.vector.tensor_copy(val2[:, :, 0], tokidf_sb[:])
                    nc.vector.tensor_copy(val2[:, :, 1], ce[:])
                    for c in range(T // 128):
                        nc.gpsimd.indirect_dma_start(
                            out=idxcb[k].ap(),
                            out_offset=bass.IndirectOffsetOnAxis(ap=oi[:, c:c + 1], axis=0),
                            in_=val2[:, c, :], in_offset=None,
                            bounds_check=A, oob_is_err=False)

                    pair = dsb.tile([128, CMAX // 128, 2], f32, tag="pair")
                    nc.sync.dma_start(
                        pair[:, 0:NCK, :],
                        idxcb[k].ap()[0:A, :].rearrange("(c p) two -> p c two", p=128))
                    idx_sb.append(persist.tile([128, NCK], i32,
                                               tag=f"idx{k}", name=f"idx{k}"))
                    nc.vector.tensor_copy(idx_sb[k][:], pair[:, 0:NCK, 0])
                    s_col.append(persist.tile([128, NCK], f32,
                                              tag=f"scol{k}", name=f"scol{k}"))
                    nc.vector.tensor_copy(s_col[k][:], pair[:, 0:NCK, 1])

                    # ---- gather routed token rows (bf16) + transpose ----
                    xt = xtp.tile([128, H // 128, CMAX], bf16, tag="xt")
                    for ck in range(NCK):
                        xg = xgp.tile([128, H], bf16, tag="xg")
                        nc.gpsimd.indirect_dma_start(
                            out=xg[:], out_offset=None,
                            in_=xbf.ap(),
                            in_offset=bass.IndirectOffsetOnAxis(
                                ap=idx_sb[k][:, ck:ck + 1], axis=0),
                            bounds_check=T - 1, oob_is_err=False)
                        for hc in range(H // 128):
                            tp = pss.tile([128, 128], bf16, tag="tp3", bufs=2)
                            nc.tensor.transpose(tp[:], xg[:, hc * 128:(hc + 1) * 128],
                                                ident_bf[:])
                            nc.vector.tensor_copy(xt[:, hc, ck * 128:(ck + 1) * 128], tp[:])

                    # ---- mm1: h^T[f, c] = gelu(sum_h W1[h,f]^T x^T[h,c] + b1[f]) ----
                    chs = [A] if A <= 512 else [A - (A // 256) * 128, (A // 256) * 128]
                    hbuf = hbp.tile([128, F // 128, CMAX], bf16, tag="hbuf")
                    for fo in range(F // 512):
                        w1t = w1p.tile([128, H // 128, 512], bf16, tag="w1t")
                        nc.scalar.dma_start(
                            w1t[:],
                            W1loc[k, :, fo * 512:(fo + 1) * 512].rearrange(
                                "(c p) f -> p c f", p=128))
                        for fi in range(4):
                            fg = fo * 4 + fi
                            cc0 = 0
                            for ch in chs:
                                ph = psh.tile([128, 512], f32, tag="ph")
                                for hc in range(H // 128):
                                    nc.tensor.matmul(
                                        ph[:, 0:ch],
                                        w1t[:, hc, fi * 128:(fi + 1) * 128],
                                        xt[:, hc, cc0:cc0 + ch],
                                        start=(hc == 0), stop=(hc == H // 128 - 1))
                                nc.scalar.activation(
                                    hbuf[:, fg, cc0:cc0 + ch], ph[:, 0:ch],
                                    Act.Gelu, bias=b1_sb[:, k, fg:fg + 1])
                                cc0 += ch

                    # ---- mm2: y[c, h] = (sum_f h^T[f,c]^T W2[f,h] + b2[h]) * s[c] ----
                    b2_sb = biasp.tile([1, H], bf16, tag="b2_sb")
                    nc.sync.dma_start(b2_sb[:], b2loc.ap()[k:k + 1, :])
                    for hh in range(2):
                        w2ts = []
                        for fgrp in range(F // 1024):
                            w2t = w2p.tile([128, 8, 512], bf16, tag="w2t",
                                           name=f"w2t{fgrp}")
                            nc.scalar.dma_start(
                                w2t[:],
                                W2loc[k, fgrp * 1024:(fgrp + 1) * 1024,
                                      hh * 512:(hh + 1) * 512].rearrange(
                                    "(c p) h -> p c h", p=128))
                            w2ts.append(w2t)
                        for ck in range(NCK):
                            pys = psy.tile([128, 512], f32, tag="py")
                            for fgrp in range(F // 1024):
                                for f8 in range(8):
                                    fg = fgrp * 8 + f8
                                    nc.tensor.matmul(
                                        pys[:],
                                        hbuf[:, fg, ck * 128:(ck + 1) * 128],
                                        w2ts[fgrp][:, f8, :],
                                        start=(fg == 0), stop=False)
                            nc.tensor.matmul(
                                pys[:], onesmm_sb[0:1, :],
                                b2_sb[0:1, hh * 512:(hh + 1) * 512],
                                start=False, stop=True)
                            ysb = ysp.tile([128, 512], f32, tag="ysb")
                            nc.vector.tensor_scalar(
                                ysb[:], pys[:], s_col[k][:, ck:ck + 1], None,
                                op0=Alu.mult)
                            nc.gpsimd.indirect_dma_start(
                                out=outp2[hh].ap(),
                                out_offset=bass.IndirectOffsetOnAxis(
                                    ap=idx_sb[k][:, ck:ck + 1], axis=0),
                                in_=ysb[:], in_offset=None,
                                compute_op=Alu.add,
                                bounds_check=T, oob_is_err=True)
                        if k == NSLOT - 1 and hh == 0:
                            # all h-half-0 contributions are in: start its
                            # ReduceScatter now so it overlaps h-half-1 compute
                            nc.gpsimd.collective_compute(
                                "ReduceScatter", Alu.add, replica_groups=RG,
                                ins=[outp2[0].ap()[0:T, :].opt()],
                                outs=[rsout2[0].ap().opt()])

            # ====== phase 4: remaining reduce + output shard ======
            with tc.tile_pool(name="outc", bufs=2) as outc:
                for k in range(TSH // 128):
                    ot = outc.tile([128, H // 2], f32, tag="ot")
                    nc.sync.dma_start(ot[:], rsout2[0][k * 128:(k + 1) * 128, :])
                    nc.sync.dma_start(out_sh[k * 128:(k + 1) * 128, 0:H // 2], ot[:])
                nc.gpsimd.collective_compute(
                    "ReduceScatter", Alu.add, replica_groups=RG,
                    ins=[outp2[1].ap()[0:T, :].opt()], outs=[rsout2[1].ap().opt()])
                for k in range(TSH // 128):
                    ot = outc.tile([128, H // 2], f32, tag="ot")
                    nc.sync.dma_start(ot[:], rsout2[1][k * 128:(k + 1) * 128, :])
                    nc.sync.dma_start(
                        out_sh[k * 128:(k + 1) * 128, H // 2:H], ot[:])

    nc.compile()
    if not nc.is_finalized():
        nc.finalize()
    return nc


def _in_maps(inputs, caps, asg):
    import ml_dtypes
    bf16 = ml_dtypes.bfloat16
    x = np.ascontiguousarray(np.asarray(inputs["x"], np.float32).reshape(T, H))
    W1 = np.asarray(inputs["W1"], np.float32)
    b1 = np.asarray(inputs["b1"], np.float32)
    W2 = np.asarray(inputs["W2"], np.float32)
    b2 = np.asarray(inputs["b2"], np.float32)
    W1b = W1.astype(bf16)
    W2b = W2.astype(bf16)
    b2b = b2.astype(bf16)
    common = {
        "xbf": np.ascontiguousarray(x.astype(bf16)),
        "Wr1": np.ascontiguousarray(np.asarray(inputs["Wr1"], np.float32)),
        "br1": np.ascontiguousarray(np.asarray(inputs["br1"], np.float32)),
        "Wr2": np.ascontiguousarray(np.asarray(inputs["Wr2"], np.float32)),
        "br2": np.ascontiguousarray(np.asarray(inputs["br2"], np.float32)),
    }
    maps = []
    for r in range(NCORES):
        w1l = np.empty((NSLOT, H, F), bf16)
        b1l = np.empty((NSLOT, F), np.float32)
        w2l = np.empty((NSLOT, F, H), bf16)
        b2l = np.empty((NSLOT, H), bf16)
        oh = np.zeros((NSLOT, E), np.float32)
        lo = np.zeros((NSLOT,), np.float32)
        for kk in range(NSLOT):
            e, l0 = asg[kk][r]
            w1l[kk] = W1b[e]
            b1l[kk] = b1[e]
            w2l[kk] = W2b[e]
            b2l[kk] = b2b[e]
            if l0 <= T:
                oh[kk, e] = 1.0     # empty slots keep an all-zero one-hot
            lo[kk] = float(l0)
        maps.append({
            **common,
            "xsh": np.ascontiguousarray(x[r * TSH:(r + 1) * TSH]),
            "W1loc": w1l, "b1loc": b1l, "W2loc": w2l, "b2loc": b2l,
            "ohloc": oh, "slotlo": lo,
        })
    return maps


def _get_nc(caps):
    key = tuple(caps)
    if key not in _CACHE:
        _CACHE[key] = _build(list(caps))
    return _CACHE[key]


def kernel(**inputs) -> np.ndarray:
    from concourse.bass_utils import run_bass_kernel_spmd

    caps, asg = _plan(inputs)
    nc = _get_nc(caps)
    maps = _in_maps(inputs, caps, asg)
    res = run_bass_kernel_spmd(nc, maps, core_ids=list(range(NCORES)))
    shards = [res.results[r]["out_sh"] for r in range(NCORES)]
    out = np.concatenate(shards, axis=0).reshape(np.asarray(inputs["x"]).shape)
    return out.astype(np.float32)


if __name__ == "__main__":
    import sys
    sys.path.insert(0, "/opt/trn_rl_repo")
    z = np.load("/root/problem/ref_cache.npz")
    inputs = {k[3:]: z[k] for k in z.files if k.startswith("in_")}
    caps, asg = _plan(inputs)
    print("caps:", caps, "sum:", sum(caps))
    used = {}
    for kk in range(NSLOT):
        for r in range(NCORES):
            e, l0 = asg[kk][r]
            if l0 <= T:
                used.setdefault(e, []).append((caps[kk], l0))
    counts = _host_counts(inputs)
    for e in sorted(used):
        cap_tot = sum(c for c, _ in used[e])
        print(f"  e{e:2d} need {counts[e]+MARGIN:5d} cap {cap_tot:5d} pieces {sorted(used[e], key=lambda p: p[1])}")
